# revision 1
# baseline (speedup 1.0000x reference)
"""Trainium2 Bass kernel for the sparse (ragged) non-local attention block.

Math (per batch b, L = lengths[b], with q/k < N=4096, c < C=256, i < CI=128):
    theta = x @ theta_w + theta_b ; phi = x @ phi_w + phi_b ; g = x @ g_w + g_b
    s[q,k] = theta[q]·phi[k]   (k >= L masked to -inf)
    attn = softmax_k(s) ; y = attn @ g ; z = (y @ W_w + W_b + x) * (q < L)

Sharding: pure data parallel — batch b on core b (8 batches, 8 cores), no
collectives. One static SPMD graph; all raggedness is folded into per-core
host-prepared inputs:
  - xt: x[b]^T in bf16 with columns k >= L zeroed.  Then phi/g columns for
    invalid keys are exactly 0 (phi_b is skipped on-chip: adding phi_b shifts
    every valid key's score by a per-query constant, which softmax cancels).
    Invalid keys thus score s=0, p=exp(0)=1, and contribute p*g=0 to y.
  - ninv = -(N-L): corrects the denominator for those exp(0)=1 terms.
  - xr = (x[b] + W_b + g_b @ W_w) * rowmask: residual with the g_b/W_b biases
    folded in exactly (sum_k attn = 1), zeroed for invalid query rows.
  - qm2: per-(row-block) validity mask, folded into the reciprocal so invalid
    rows emit 0.

On-chip per core (all matmuls bf16, f32 PSUM accumulation):
  thetaT/phiT [ci,n] projections (theta_b added per-partition), g [k,ci]
  natural-layout projection; then per 512-query chunk: for each 128-key block
  s^T[k,q] = phiT_kb^T @ thetaT (PE) -> p = exp(s) (ScalarE, bf16) ->
  yT[ci,q] += g_kb^T @ p (PE, PSUM accum) and psb[slice] += p (DVE, bf16);
  denom = sum_slices ones^T @ psb (PE, f32 accum) + ninv; r = qm/denom
  (partition-parallel after a 1->128 spread DMA); per 128-query block
  w = yT_qb^T @ W_w (PE), out = w * r + xr (one fused DVE op) -> DMA out.
"""

import sys

if "/opt/trn_rl_repo" not in sys.path:
    sys.path.insert(0, "/opt/trn_rl_repo")

import contextlib
import ctypes
import types

import ml_dtypes
import numpy as np

import concourse.bass as bass
import concourse.mybir as mybir
import concourse.tile as tile
from concourse import bacc
from concourse.bass import ts

B, N, C, CI = 8, 4096, 256, 128
P = 128
NKB = N // P  # 32 key blocks
QC = 512  # query chunk
NQC = N // QC  # 8
GSZ = 3  # key blocks per exp group (3 PSUM banks wide)
NSL = 4  # bf16 p_sum slices (absorption control)

dt = mybir.dt
AF = mybir.ActivationFunctionType
OP = mybir.AluOpType

LAST_EXEC_NS = None


def _install_ntff_shim():
    """Register the axon NTFF profile hook (missing antenv.axon_hooks in this
    image) so run_bass_kernel_spmd(trace=True) can report HW exec time."""
    if "antenv.axon_hooks" in sys.modules:
        return
    try:
        import antenv

        mod = types.ModuleType("antenv.axon_hooks")
        _state = {"hook": None}
        mod.set_axon_ntff_profile_hook = lambda h: _state.__setitem__("hook", h)
        mod.get_axon_ntff_profile_hook = lambda: _state["hook"]
        sys.modules["antenv.axon_hooks"] = mod
        antenv.axon_hooks = mod

        lib = ctypes.CDLL("/opt/axon/libaxon_pjrt.so")
        if not hasattr(lib, "axon_start_nrt_profile"):
            return
        lib.axon_start_nrt_profile.argtypes = [
            ctypes.POINTER(ctypes.c_int64),
            ctypes.c_size_t,
        ]
        lib.axon_start_nrt_profile.restype = ctypes.c_int64
        lib.axon_stop_nrt_profile.argtypes = [ctypes.c_char_p]
        lib.axon_stop_nrt_profile.restype = ctypes.c_int64

        @contextlib.contextmanager
        def _hook(output_dir, device_ids):
            import jax

            jax.devices()
            if device_ids:
                ids = (ctypes.c_int64 * len(device_ids))(*device_ids)
                rc = lib.axon_start_nrt_profile(ids, len(device_ids))
            else:
                rc = lib.axon_start_nrt_profile(None, 0)
            if rc != 0:
                raise RuntimeError(f"axon_start_nrt_profile rc={rc}")
            try:
                yield
            finally:
                n = lib.axon_stop_nrt_profile(str(output_dir).encode())
                if n < 0:
                    raise RuntimeError(f"axon_stop_nrt_profile rc={n}")

        mod.set_axon_ntff_profile_hook(_hook)
    except Exception:
        pass


def _enable_ldw_opt():
    """Flip walrus --enable-ldw-opt to true (overlaps LDWEIGHTS with matmul
    streaming via the background weight buffer)."""
    from concourse import bass_utils as bu

    if getattr(bu, "_ldw_patched", False):
        return
    orig = bu.run_command

    def patched(cmd, *a, **kw):
        if isinstance(cmd, list):
            cmd = [
                "--enable-ldw-opt=true" if c == "--enable-ldw-opt=false" else c
                for c in cmd
            ]
        return orig(cmd, *a, **kw)

    bu.run_command = patched
    bu._ldw_patched = True


def build(nkb):
    # nkb = number of 128-key blocks actually processed (= max over cores of
    # ceil(L/128)); blocks beyond it are fully masked on every core and the
    # host denominator correction counts only the processed keys.
    groups = []
    _kb = 0
    while _kb < nkb:
        g = min(GSZ, nkb - _kb)
        groups.append((_kb, g))
        _kb += g
    kb_per_sl = max(1, -(-nkb // NSL))
    slices_used = -(-nkb // kb_per_sl)
    nc = bacc.Bacc("TRN2", target_bir_lowering=False, debug=False, num_devices=B)

    xt = nc.declare_dram_parameter("xt", [2, P, N], dt.bfloat16, False)
    xr = nc.declare_dram_parameter("xr", [N, C], dt.float32, False)
    tw = nc.declare_dram_parameter("tw", [2, P, CI], dt.bfloat16, False)
    pw = nc.declare_dram_parameter("pw", [2, P, CI], dt.bfloat16, False)
    gw = nc.declare_dram_parameter("gw", [2, P, CI], dt.bfloat16, False)
    ww = nc.declare_dram_parameter("ww", [CI, C], dt.bfloat16, False)
    tb = nc.declare_dram_parameter("tb", [P, 1], dt.float32, False)
    qm2 = nc.declare_dram_parameter("qm2", [P, NKB], dt.float32, False)
    ninv = nc.declare_dram_parameter("ninv", [P, 1], dt.float32, False)
    out = nc.declare_dram_parameter("out", [N, C], dt.float32, True)

    with tile.TileContext(nc) as tc:
        with (
            tc.tile_pool(name="wpool", bufs=1) as wpool,
            tc.tile_pool(name="xtp", bufs=1) as xtp,
            tc.tile_pool(name="feat", bufs=1) as feat,
            tc.tile_pool(name="ppool", bufs=3) as ppool,
            tc.tile_pool(name="psbp", bufs=2) as psbp,
            tc.tile_pool(name="ysbp", bufs=2) as ysbp,
            tc.tile_pool(name="smallp", bufs=2) as smallp,
            tc.tile_pool(name="xrp", bufs=8) as xrp,
            tc.tile_pool(name="outp", bufs=4) as outp,
            tc.tile_pool(name="sc_ps", bufs=2, space="PSUM") as sc_ps,
            tc.tile_pool(name="y_ps", bufs=2, space="PSUM") as y_ps,
        ):
            # ---- constants / weights to SBUF ----
            tw_s = wpool.tile([P, 2 * CI], dt.bfloat16, tag="tw")
            pw_s = wpool.tile([P, 2 * CI], dt.bfloat16, tag="pw")
            gw_s = wpool.tile([P, 2 * CI], dt.bfloat16, tag="gw")
            for i in range(2):
                nc.sync.dma_start(tw_s[:, ts(i, CI)], tw.ap()[i])
                nc.sync.dma_start(pw_s[:, ts(i, CI)], pw.ap()[i])
                nc.sync.dma_start(gw_s[:, ts(i, CI)], gw.ap()[i])
            ww_s = wpool.tile([CI, C], dt.bfloat16, tag="ww")
            nc.sync.dma_start(ww_s[:], ww.ap()[:])
            tb_s = wpool.tile([P, 1], dt.float32, tag="tb")
            nc.sync.dma_start(tb_s[:], tb.ap()[:])
            qm_s = wpool.tile([P, NKB], dt.float32, tag="qm")
            nc.sync.dma_start(qm_s[:], qm2.ap()[:])
            ninv_s = wpool.tile([P, 1], dt.float32, tag="ninv")
            nc.sync.dma_start(ninv_s[:], ninv.ap()[:])
            ones_s = wpool.tile([P, 1], dt.bfloat16, tag="ones")
            nc.vector.memset(ones_s[:], 1.0)
            one_f = wpool.tile([1, 1], dt.float32, tag="onef")
            nc.vector.memset(one_f[:], 1.0)

            xt_s = xtp.tile([P, 2 * N], dt.bfloat16, tag="xt")
            for i in range(2):
                nc.sync.dma_start(xt_s[:, ts(i, N)], xt.ap()[i])

            # ---- projections ----
            thetaT = feat.tile([P, N], dt.bfloat16, tag="thetaT")
            phiT = feat.tile([P, N], dt.bfloat16, tag="phiT")
            g_s = feat.tile([P, N], dt.bfloat16, tag="g")

            nph = -(-nkb * P // QC)  # phi chunks needed to cover nkb key blocks
            for ch in range(NQC):
                pth = sc_ps.tile([P, GSZ, QC], dt.float32, tag="sc", name="pth")
                nc.tensor.matmul(
                    pth[:, 0, :],
                    lhsT=tw_s[:, 0:CI],
                    rhs=xt_s[:, ch * QC : (ch + 1) * QC],
                    start=True,
                    stop=False,
                )
                nc.tensor.matmul(
                    pth[:, 0, :],
                    lhsT=tw_s[:, CI : 2 * CI],
                    rhs=xt_s[:, N + ch * QC : N + (ch + 1) * QC],
                    start=False,
                    stop=True,
                )
                nc.vector.tensor_scalar_add(
                    thetaT[:, ts(ch, QC)], pth[:, 0, :], tb_s[:, 0:1]
                )
                if ch >= nph:
                    continue
                pph = sc_ps.tile([P, GSZ, QC], dt.float32, tag="sc", name="pph")
                nc.tensor.matmul(
                    pph[:, 0, :],
                    lhsT=pw_s[:, 0:CI],
                    rhs=xt_s[:, ch * QC : (ch + 1) * QC],
                    start=True,
                    stop=False,
                )
                nc.tensor.matmul(
                    pph[:, 0, :],
                    lhsT=pw_s[:, CI : 2 * CI],
                    rhs=xt_s[:, N + ch * QC : N + (ch + 1) * QC],
                    start=False,
                    stop=True,
                )
                nc.scalar.copy(phiT[:, ts(ch, QC)], pph[:, 0, :])

            for kbp in range(-(-nkb // 2)):
                pg = sc_ps.tile([P, GSZ, QC], dt.float32, tag="sc", name="pg")
                for h in range(2):
                    kb = kbp * 2 + h
                    nc.tensor.matmul(
                        pg[:, h, 0:CI],
                        lhsT=xt_s[:, kb * P : (kb + 1) * P],
                        rhs=gw_s[:, 0:CI],
                        start=True,
                        stop=False,
                    )
                    nc.tensor.matmul(
                        pg[:, h, 0:CI],
                        lhsT=xt_s[:, N + kb * P : N + (kb + 1) * P],
                        rhs=gw_s[:, CI : 2 * CI],
                        start=False,
                        stop=True,
                    )
                nc.scalar.copy(
                    g_s[:, kbp * 2 * P : (kbp + 1) * 2 * P].rearrange(
                        "p (h c) -> p h c", h=2
                    ),
                    pg[:, 0:2, 0:CI],
                )

            # ---- attention, software-pipelined one exp-group deep ----
            qstate = {}

            def start_qc(qc):
                ysum = y_ps.tile([P, QC], dt.float32, tag="ysum", name="ysum")
                psb = psbp.tile([P, NSL, QC], dt.bfloat16, tag="psb", name="psb")
                xr_ts = []
                for j in range(4):
                    qb = qc * 4 + j
                    xr_t = xrp.tile([P, C], dt.float32, tag="xr", name="xr_t")
                    nc.sync.dma_start(xr_t[:], xr.ap()[qb * P : (qb + 1) * P, :])
                    xr_ts.append(xr_t)
                qstate[qc] = (ysum, psb, xr_ts)

            def drain(item):
                qc, kb0, gsz, p = item
                if kb0 == 0:
                    start_qc(qc)
                ysum, psb, xr_ts = qstate[qc]
                for j in range(gsz):
                    kbj = kb0 + j
                    nc.tensor.matmul(
                        ysum[:],
                        lhsT=g_s[:, ts(kbj, P)],
                        rhs=p[:, j, :],
                        start=(kbj == 0),
                        stop=(kbj == nkb - 1),
                        skip_group_check=True,
                    )
                    sl = kbj // kb_per_sl
                    if kbj % kb_per_sl == 0:
                        nc.vector.tensor_copy(psb[:, sl, :], p[:, j, :])
                    else:
                        nc.vector.tensor_add(psb[:, sl, :], psb[:, sl, :], p[:, j, :])
                if kb0 + gsz == nkb:
                    finish_queue.append([qc, 0])

            def finish_qc(qc):
                ysum, psb, xr_ts = qstate.pop(qc)
                ds = sc_ps.tile([P, GSZ, QC], dt.float32, tag="sc", name="ds")
                for sl in range(slices_used):
                    nc.tensor.matmul(
                        ds[0:1, 0, :],
                        lhsT=ones_s[:, 0:1],
                        rhs=psb[:, sl, :],
                        start=(sl == 0),
                        stop=(sl == slices_used - 1),
                        skip_group_check=True,
                    )
                ds_sb = smallp.tile([1, QC], dt.float32, tag="ds_sb", name="ds_sb")
                nc.vector.tensor_copy(ds_sb[:], ds[0:1, 0, :])
                dsp = sc_ps.tile([P, GSZ, QC], dt.float32, tag="sc", name="dsp")
                for j in range(4):
                    nc.tensor.matmul(
                        dsp[:, 0, j : j + 1],
                        lhsT=ds_sb[0:1, ts(j, P)],
                        rhs=one_f[0:1, 0:1],
                        start=(j == 0),
                        stop=(j == 3),
                        skip_group_check=True,
                    )
                dn = smallp.tile([P, 4], dt.float32, tag="dn", name="dn")
                nc.vector.tensor_scalar_add(dn[:], dsp[:, 0, 0:4], ninv_s[:, 0:1])
                rc = smallp.tile([P, 4], dt.float32, tag="rc", name="rc")
                nc.vector.reciprocal(rc[:], dn[:])
                r_t = smallp.tile([P, 4], dt.float32, tag="rt", name="r_t")
                nc.vector.tensor_mul(
                    r_t[:], rc[:], qm_s[:, qc * 4 : (qc + 1) * 4]
                )
                y_sb = ysbp.tile([P, QC], dt.bfloat16, tag="ysb", name="y_sb")
                nc.vector.tensor_copy(y_sb[:], ysum[:])
                for j in range(4):
                    qb = qc * 4 + j
                    wy = sc_ps.tile([P, GSZ, QC], dt.float32, tag="sc", name="wy")
                    nc.tensor.matmul(
                        wy[:, 0, 0:C],
                        lhsT=y_sb[:, ts(j, P)],
                        rhs=ww_s[:],
                        start=True,
                        stop=True,
                    )
                    ot = outp.tile([P, C], dt.float32, tag="ot", name="ot")
                    nc.vector.scalar_tensor_tensor(
                        ot[:],
                        wy[:, 0, 0:C],
                        r_t[:, j : j + 1],
                        xr_ts[j][:],
                        OP.mult,
                        OP.add,
                    )
                    nc.sync.dma_start(out.ap()[qb * P : (qb + 1) * P, :], ot[:])

            pending = []
            finish_queue = []

            def tick_finishes(force=False):
                for ent in list(finish_queue):
                    ent[1] += 1
                    if force or ent[1] > 2:
                        finish_qc(ent[0])
                        finish_queue.remove(ent)

            for qc in range(NQC):
                for kb0, gsz in groups:
                    sc = sc_ps.tile([P, GSZ, QC], dt.float32, tag="sc", name="sc")
                    for j in range(gsz):
                        nc.tensor.matmul(
                            sc[:, j, :],
                            lhsT=phiT[:, ts(kb0 + j, P)],
                            rhs=thetaT[:, qc * QC : (qc + 1) * QC],
                            start=True,
                            stop=True,
                        )
                    p = ppool.tile([P, GSZ, QC], dt.bfloat16, tag="p", name="p")
                    nc.scalar.activation(p[:, :gsz, :], sc[:, :gsz, :], AF.Exp)
                    pending.append((qc, kb0, gsz, p))
                    if len(pending) > 1:
                        drain(pending.pop(0))
                        tick_finishes()
            while pending:
                drain(pending.pop(0))
            tick_finishes(force=True)

    nc.compile()
    return nc


_NC_CACHE = {}


def kernel(**inputs):
    global LAST_EXEC_NS
    _install_ntff_shim()
    from concourse.bass_utils import run_bass_kernel_spmd

    x = np.asarray(inputs["x"], dtype=np.float32)
    lengths = np.asarray(inputs["lengths"]).astype(np.int64)
    theta_w = np.asarray(inputs["theta_w"], np.float32)
    theta_b = np.asarray(inputs["theta_b"], np.float32)
    phi_w = np.asarray(inputs["phi_w"], np.float32)
    g_w = np.asarray(inputs["g_w"], np.float32)
    g_b = np.asarray(inputs["g_b"], np.float32)
    W_w = np.asarray(inputs["W_w"], np.float32)
    W_b = np.asarray(inputs["W_b"], np.float32)

    bf16 = ml_dtypes.bfloat16
    tw_np = np.ascontiguousarray(theta_w.reshape(2, P, CI)).astype(bf16)
    pw_np = np.ascontiguousarray(phi_w.reshape(2, P, CI)).astype(bf16)
    gw_np = np.ascontiguousarray(g_w.reshape(2, P, CI)).astype(bf16)
    ww_np = np.ascontiguousarray(W_w).astype(bf16)
    tb_np = np.ascontiguousarray(theta_b.reshape(P, 1)).astype(np.float32)
    resid_base = (W_b + g_b @ W_w)[None, :].astype(np.float32)

    lens = [max(0, min(N, int(lengths[b]))) for b in range(B)]
    nkb = max(1, max(-(-L // P) for L in lens))
    keys_processed = nkb * P
    in_maps = []
    for b in range(B):
        L = lens[b]
        rowmask = (np.arange(N) < L).astype(np.float32)
        xz = x[b] * rowmask[:, None]
        xt_np = np.ascontiguousarray(xz.T).reshape(2, P, N).astype(bf16)
        xr_np = np.ascontiguousarray((x[b] + resid_base) * rowmask[:, None]).astype(
            np.float32
        )
        ninv_val = -(keys_processed - L) + (1.0 if L == 0 else 0.0)
        qm2_np = np.ascontiguousarray(rowmask.reshape(NKB, P).T)
        in_maps.append(
            {
                "xt": xt_np,
                "xr": xr_np,
                "tw": tw_np,
                "pw": pw_np,
                "gw": gw_np,
                "ww": ww_np,
                "tb": tb_np,
                "qm2": qm2_np,
                "ninv": np.full((P, 1), ninv_val, np.float32),
            }
        )

    if nkb not in _NC_CACHE:
        _NC_CACHE[nkb] = build(nkb)
    nc = _NC_CACHE[nkb]

    res = run_bass_kernel_spmd(nc, in_maps, list(range(B)))
    LAST_EXEC_NS = res.exec_time_ns
    out = np.stack([np.asarray(res.results[i]["out"]) for i in range(B)]).astype(
        np.float32
    )
    return out


if __name__ == "__main__":
    rng = np.random.default_rng(0)
    demo = {
        "x": rng.standard_normal((B, N, C), dtype=np.float32),
        "lengths": rng.integers(N // 2, N + 1, size=(B,)).astype(np.int32),
        "g_w": (rng.standard_normal((C, CI)) * 0.02).astype(np.float32),
        "g_b": np.zeros(CI, np.float32),
        "theta_w": (rng.standard_normal((C, CI)) * 0.02).astype(np.float32),
        "theta_b": np.zeros(CI, np.float32),
        "phi_w": (rng.standard_normal((C, CI)) * 0.02).astype(np.float32),
        "phi_b": np.zeros(CI, np.float32),
        "W_w": (rng.standard_normal((CI, C)) * 0.02).astype(np.float32),
        "W_b": np.zeros(C, np.float32),
    }
    o = kernel(**demo)
    print("out", o.shape, o.dtype, float(np.abs(o).mean()))



# revision 3
# speedup vs baseline: 1.0318x; 1.0318x over previous
"""Trainium2 Bass kernel for the sparse (ragged) non-local attention block.

Math (per batch b, L = lengths[b], with q/k < N=4096, c < C=256, i < CI=128):
    theta = x @ theta_w + theta_b ; phi = x @ phi_w + phi_b ; g = x @ g_w + g_b
    s[q,k] = theta[q]·phi[k]   (k >= L masked to -inf)
    attn = softmax_k(s) ; y = attn @ g ; z = (y @ W_w + W_b + x) * (q < L)

Sharding: pure data parallel — batch b on core b (8 batches, 8 cores), no
collectives. One static SPMD graph; all raggedness is folded into per-core
host-prepared inputs:
  - xt: x[b]^T in bf16 with columns k >= L zeroed.  Then phi/g columns for
    invalid keys are exactly 0 (phi_b is skipped on-chip: adding phi_b shifts
    every valid key's score by a per-query constant, which softmax cancels).
    Invalid keys thus score s=0, p=exp(0)=1, and contribute p*g=0 to y.
  - ninv = -(N-L): corrects the denominator for those exp(0)=1 terms.
  - xr = (x[b] + W_b + g_b @ W_w) * rowmask: residual with the g_b/W_b biases
    folded in exactly (sum_k attn = 1), zeroed for invalid query rows.
  - qm2: per-(row-block) validity mask, folded into the reciprocal so invalid
    rows emit 0.

Only the first nkb = max_b ceil(L_b/128) query blocks are processed (rows
q >= nkb*128 are masked to zero on every core anyway); the tail output rows
are zero-filled by DMA from a memset tile.

On-chip per core (all matmuls bf16, f32 PSUM accumulation):
  thetaT/phiT [ci,n] projections (theta_b added per-partition), g [k,ci]
  natural-layout projection; then per query chunk (up to 512 wide): for each
  128-key block s^T[k,q] = phiT_kb^T @ thetaT (PE) -> p = exp(s) (ScalarE,
  bf16) -> yT[ci,q] += g_kb^T @ p (PE, PSUM accum) and psb += p (DVE, bf16);
  denom = ones^T @ psb (PE, f32 accum) + ninv; r = qm/denom
  (partition-parallel after a 1->128 spread matmul); per 128-query block
  w = yT_qb^T @ W_w (PE), out = w * r + xr (one fused DVE op) -> DMA out.
"""

import sys

if "/opt/trn_rl_repo" not in sys.path:
    sys.path.insert(0, "/opt/trn_rl_repo")

import contextlib
import ctypes
import types

import ml_dtypes
import numpy as np

import concourse.bass as bass
import concourse.mybir as mybir
import concourse.tile as tile
from concourse import bacc
from concourse.bass import ts

B, N, C, CI = 8, 4096, 256, 128
P = 128
NKB = N // P  # 32 key blocks
QC = 512  # query chunk
GSZ = 3  # key blocks per exp group (3 PSUM banks wide)
NSL = 1  # bf16 p_sum slices

dt = mybir.dt
AF = mybir.ActivationFunctionType
OP = mybir.AluOpType

LAST_EXEC_NS = None


def _install_ntff_shim():
    """Register the axon NTFF profile hook (missing antenv.axon_hooks in this
    image) so run_bass_kernel_spmd(trace=True) can report HW exec time."""
    if "antenv.axon_hooks" in sys.modules:
        return
    try:
        import antenv

        mod = types.ModuleType("antenv.axon_hooks")
        _state = {"hook": None}
        mod.set_axon_ntff_profile_hook = lambda h: _state.__setitem__("hook", h)
        mod.get_axon_ntff_profile_hook = lambda: _state["hook"]
        sys.modules["antenv.axon_hooks"] = mod
        antenv.axon_hooks = mod

        lib = ctypes.CDLL("/opt/axon/libaxon_pjrt.so")
        if not hasattr(lib, "axon_start_nrt_profile"):
            return
        lib.axon_start_nrt_profile.argtypes = [
            ctypes.POINTER(ctypes.c_int64),
            ctypes.c_size_t,
        ]
        lib.axon_start_nrt_profile.restype = ctypes.c_int64
        lib.axon_stop_nrt_profile.argtypes = [ctypes.c_char_p]
        lib.axon_stop_nrt_profile.restype = ctypes.c_int64

        @contextlib.contextmanager
        def _hook(output_dir, device_ids):
            import jax

            jax.devices()
            if device_ids:
                ids = (ctypes.c_int64 * len(device_ids))(*device_ids)
                rc = lib.axon_start_nrt_profile(ids, len(device_ids))
            else:
                rc = lib.axon_start_nrt_profile(None, 0)
            if rc != 0:
                raise RuntimeError(f"axon_start_nrt_profile rc={rc}")
            try:
                yield
            finally:
                n = lib.axon_stop_nrt_profile(str(output_dir).encode())
                if n < 0:
                    raise RuntimeError(f"axon_stop_nrt_profile rc={n}")

        mod.set_axon_ntff_profile_hook(_hook)
    except Exception:
        pass


def _enable_ldw_opt():
    """Flip walrus --enable-ldw-opt to true (overlaps LDWEIGHTS with matmul
    streaming via the background weight buffer)."""
    from concourse import bass_utils as bu

    if getattr(bu, "_ldw_patched", False):
        return
    orig = bu.run_command

    def patched(cmd, *a, **kw):
        if isinstance(cmd, list):
            cmd = [
                "--enable-ldw-opt=true" if c == "--enable-ldw-opt=false" else c
                for c in cmd
            ]
        return orig(cmd, *a, **kw)

    bu.run_command = patched
    bu._ldw_patched = True


def build(nkb):
    # nkb = number of 128-wide blocks actually processed on both the key and
    # the query axis (= max over cores of ceil(L/128)); key blocks beyond it
    # are fully masked on every core (host denominator correction counts only
    # processed keys) and query rows beyond it are zero on every core.
    groups = []
    _kb = 0
    while _kb < nkb:
        g = min(GSZ, nkb - _kb)
        groups.append((_kb, g))
        _kb += g
    # query chunks: (start_block, n_blocks)
    chunks = []
    _qb = 0
    while _qb < nkb:
        nb = min(QC // P, nkb - _qb)
        chunks.append((_qb, nb))
        _qb += nb
    kb_per_sl = max(1, -(-nkb // NSL))
    slices_used = -(-nkb // kb_per_sl)
    nc = bacc.Bacc("TRN2", target_bir_lowering=False, debug=False, num_devices=B)

    xt = nc.declare_dram_parameter("xt", [2, P, N], dt.bfloat16, False)
    xr = nc.declare_dram_parameter("xr", [N, C], dt.float32, False)
    tw = nc.declare_dram_parameter("tw", [2, P, CI], dt.bfloat16, False)
    pw = nc.declare_dram_parameter("pw", [2, P, CI], dt.bfloat16, False)
    gw = nc.declare_dram_parameter("gw", [2, P, CI], dt.bfloat16, False)
    ww = nc.declare_dram_parameter("ww", [CI, C], dt.bfloat16, False)
    tb = nc.declare_dram_parameter("tb", [P, 1], dt.float32, False)
    qm2 = nc.declare_dram_parameter("qm2", [P, NKB], dt.float32, False)
    ninv = nc.declare_dram_parameter("ninv", [P, 1], dt.float32, False)
    out = nc.declare_dram_parameter("out", [N, C], dt.float32, True)

    nq = nkb * P  # processed queries / keys

    with tile.TileContext(nc) as tc:
        with (
            tc.tile_pool(name="wpool", bufs=1) as wpool,
            tc.tile_pool(name="xtp", bufs=1) as xtp,
            tc.tile_pool(name="feat", bufs=1) as feat,
            tc.tile_pool(name="ppool", bufs=3) as ppool,
            tc.tile_pool(name="psbp", bufs=2) as psbp,
            tc.tile_pool(name="ysbp", bufs=2) as ysbp,
            tc.tile_pool(name="smallp", bufs=2) as smallp,
            tc.tile_pool(name="xrp", bufs=8) as xrp,
            tc.tile_pool(name="outp", bufs=4) as outp,
            tc.tile_pool(name="sc_ps", bufs=2, space="PSUM") as sc_ps,
            tc.tile_pool(name="y_ps", bufs=2, space="PSUM") as y_ps,
        ):
            # ---- constants / weights to SBUF ----
            tw_s = wpool.tile([P, 2 * CI], dt.bfloat16, tag="tw")
            pw_s = wpool.tile([P, 2 * CI], dt.bfloat16, tag="pw")
            gw_s = wpool.tile([P, 2 * CI], dt.bfloat16, tag="gw")
            for i in range(2):
                nc.sync.dma_start(tw_s[:, ts(i, CI)], tw.ap()[i])
                nc.sync.dma_start(pw_s[:, ts(i, CI)], pw.ap()[i])
                nc.sync.dma_start(gw_s[:, ts(i, CI)], gw.ap()[i])
            ww_s = wpool.tile([CI, C], dt.bfloat16, tag="ww")
            nc.sync.dma_start(ww_s[:], ww.ap()[:])
            tb_s = wpool.tile([P, 1], dt.float32, tag="tb")
            nc.sync.dma_start(tb_s[:], tb.ap()[:])
            qm_s = wpool.tile([P, NKB], dt.float32, tag="qm")
            nc.sync.dma_start(qm_s[:], qm2.ap()[:])
            ninv_s = wpool.tile([P, 1], dt.float32, tag="ninv")
            nc.sync.dma_start(ninv_s[:], ninv.ap()[:])
            ones_s = wpool.tile([P, 1], dt.bfloat16, tag="ones")
            nc.vector.memset(ones_s[:], 1.0)
            one_f = wpool.tile([1, 1], dt.float32, tag="onef")
            nc.vector.memset(one_f[:], 1.0)

            # zero-fill tile for unprocessed query rows
            if nkb < NKB:
                z_s = wpool.tile([P, C], dt.float32, tag="z")
                nc.vector.memset(z_s[:], 0.0)
                for qb in range(nkb, NKB):
                    nc.sync.dma_start(out.ap()[qb * P : (qb + 1) * P, :], z_s[:])

            xt_s = xtp.tile([P, 2 * N], dt.bfloat16, tag="xt")
            for i in range(2):
                nc.sync.dma_start(xt_s[:, ts(i, N)], xt.ap()[i])

            # ---- projections (only the first nq columns are needed) ----
            thetaT = feat.tile([P, N], dt.bfloat16, tag="thetaT")
            phiT = feat.tile([P, N], dt.bfloat16, tag="phiT")
            g_s = feat.tile([P, N], dt.bfloat16, tag="g")

            for ch, (qb0, nb) in enumerate(chunks):
                w = nb * P
                c0 = qb0 * P
                pth = sc_ps.tile([P, GSZ, QC], dt.float32, tag="sc", name="pth")
                nc.tensor.matmul(
                    pth[:, 0, :w],
                    lhsT=tw_s[:, 0:CI],
                    rhs=xt_s[:, c0 : c0 + w],
                    start=True,
                    stop=False,
                )
                nc.tensor.matmul(
                    pth[:, 0, :w],
                    lhsT=tw_s[:, CI : 2 * CI],
                    rhs=xt_s[:, N + c0 : N + c0 + w],
                    start=False,
                    stop=True,
                )
                nc.vector.tensor_scalar_add(
                    thetaT[:, c0 : c0 + w], pth[:, 0, :w], tb_s[:, 0:1]
                )
                pph = sc_ps.tile([P, GSZ, QC], dt.float32, tag="sc", name="pph")
                nc.tensor.matmul(
                    pph[:, 0, :w],
                    lhsT=pw_s[:, 0:CI],
                    rhs=xt_s[:, c0 : c0 + w],
                    start=True,
                    stop=False,
                )
                nc.tensor.matmul(
                    pph[:, 0, :w],
                    lhsT=pw_s[:, CI : 2 * CI],
                    rhs=xt_s[:, N + c0 : N + c0 + w],
                    start=False,
                    stop=True,
                )
                nc.vector.tensor_copy(phiT[:, c0 : c0 + w], pph[:, 0, :w])

            for kbp in range(-(-nkb // 2)):
                pg = sc_ps.tile([P, GSZ, QC], dt.float32, tag="sc", name="pg")
                nh = min(2, nkb - kbp * 2)
                for h in range(nh):
                    kb = kbp * 2 + h
                    nc.tensor.matmul(
                        pg[:, h, 0:CI],
                        lhsT=xt_s[:, kb * P : (kb + 1) * P],
                        rhs=gw_s[:, 0:CI],
                        start=True,
                        stop=False,
                    )
                    nc.tensor.matmul(
                        pg[:, h, 0:CI],
                        lhsT=xt_s[:, N + kb * P : N + (kb + 1) * P],
                        rhs=gw_s[:, CI : 2 * CI],
                        start=False,
                        stop=True,
                    )
                nc.vector.tensor_copy(
                    g_s[:, kbp * 2 * P : kbp * 2 * P + nh * P].rearrange(
                        "p (h c) -> p h c", h=nh
                    ),
                    pg[:, 0:nh, 0:CI],
                )

            # ---- attention, software-pipelined one exp-group deep ----
            qstate = {}

            def start_qc(qc):
                qb0, nb = chunks[qc]
                ysum = y_ps.tile([P, QC], dt.float32, tag="ysum", name="ysum")
                psb = psbp.tile([P, NSL, QC], dt.bfloat16, tag="psb", name="psb")
                xr_ts = []
                for j in range(nb):
                    qb = qb0 + j
                    xr_t = xrp.tile([P, C], dt.float32, tag="xr", name="xr_t")
                    nc.sync.dma_start(xr_t[:], xr.ap()[qb * P : (qb + 1) * P, :])
                    xr_ts.append(xr_t)
                qstate[qc] = (ysum, psb, xr_ts)

            def drain(item):
                qc, kb0, gsz, p = item
                qb0, nb = chunks[qc]
                w = nb * P
                if kb0 == 0:
                    start_qc(qc)
                ysum, psb, xr_ts = qstate[qc]
                for j in range(gsz):
                    kbj = kb0 + j
                    nc.tensor.matmul(
                        ysum[:, :w],
                        lhsT=g_s[:, ts(kbj, P)],
                        rhs=p[:, j, :w],
                        start=(kbj == 0),
                        stop=(kbj == nkb - 1),
                        skip_group_check=True,
                    )
                    sl = kbj // kb_per_sl
                    if kbj % kb_per_sl == 0:
                        nc.vector.tensor_copy(psb[:, sl, :w], p[:, j, :w])
                    else:
                        nc.vector.tensor_add(
                            psb[:, sl, :w], psb[:, sl, :w], p[:, j, :w]
                        )
                if kb0 + gsz == nkb:
                    finish_queue.append([qc, 0])

            def finish_qc(qc):
                qb0, nb = chunks[qc]
                w = nb * P
                ysum, psb, xr_ts = qstate.pop(qc)
                ds = sc_ps.tile([P, GSZ, QC], dt.float32, tag="sc", name="ds")
                for sl in range(slices_used):
                    nc.tensor.matmul(
                        ds[0:1, 0, :w],
                        lhsT=ones_s[:, 0:1],
                        rhs=psb[:, sl, :w],
                        start=(sl == 0),
                        stop=(sl == slices_used - 1),
                        skip_group_check=True,
                    )
                ds_sb = smallp.tile([1, QC], dt.float32, tag="ds_sb", name="ds_sb")
                nc.vector.tensor_copy(ds_sb[0:1, :w], ds[0:1, 0, :w])
                dsp = sc_ps.tile([P, GSZ, QC], dt.float32, tag="sc", name="dsp")
                for j in range(nb):
                    nc.tensor.matmul(
                        dsp[:, 0, j : j + 1],
                        lhsT=ds_sb[0:1, ts(j, P)],
                        rhs=one_f[0:1, 0:1],
                        start=(j == 0),
                        stop=(j == nb - 1),
                        skip_group_check=True,
                    )
                dn = smallp.tile([P, 4], dt.float32, tag="dn", name="dn")
                nc.vector.tensor_scalar_add(
                    dn[:, :nb], dsp[:, 0, 0:nb], ninv_s[:, 0:1]
                )
                rc = smallp.tile([P, 4], dt.float32, tag="rc", name="rc")
                nc.vector.reciprocal(rc[:, :nb], dn[:, :nb])
                r_t = smallp.tile([P, 4], dt.float32, tag="rt", name="r_t")
                nc.vector.tensor_mul(
                    r_t[:, :nb], rc[:, :nb], qm_s[:, qb0 : qb0 + nb]
                )
                y_sb = ysbp.tile([P, QC], dt.bfloat16, tag="ysb", name="y_sb")
                nc.vector.tensor_copy(y_sb[:, :w], ysum[:, :w])
                for j in range(nb):
                    qb = qb0 + j
                    wy = sc_ps.tile([P, GSZ, QC], dt.float32, tag="sc", name="wy")
                    nc.tensor.matmul(
                        wy[:, 0, 0:C],
                        lhsT=y_sb[:, ts(j, P)],
                        rhs=ww_s[:],
                        start=True,
                        stop=True,
                    )
                    ot = outp.tile([P, C], dt.float32, tag="ot", name="ot")
                    nc.vector.scalar_tensor_tensor(
                        ot[:],
                        wy[:, 0, 0:C],
                        r_t[:, j : j + 1],
                        xr_ts[j][:],
                        OP.mult,
                        OP.add,
                    )
                    nc.sync.dma_start(out.ap()[qb * P : (qb + 1) * P, :], ot[:])

            pending = []
            finish_queue = []

            def tick_finishes(force=False):
                for ent in list(finish_queue):
                    ent[1] += 1
                    if force or ent[1] > 2:
                        finish_qc(ent[0])
                        finish_queue.remove(ent)

            for qc, (qb0, nb) in enumerate(chunks):
                w = nb * P
                c0 = qb0 * P
                for kb0, gsz in groups:
                    sc = sc_ps.tile([P, GSZ, QC], dt.float32, tag="sc", name="sc")
                    for j in range(gsz):
                        nc.tensor.matmul(
                            sc[:, j, :w],
                            lhsT=phiT[:, ts(kb0 + j, P)],
                            rhs=thetaT[:, c0 : c0 + w],
                            start=True,
                            stop=True,
                        )
                    p = ppool.tile([P, GSZ, QC], dt.bfloat16, tag="p", name="p")
                    nc.scalar.activation(p[:, :gsz, :w], sc[:, :gsz, :w], AF.Exp)
                    pending.append((qc, kb0, gsz, p))
                    if len(pending) > 1:
                        drain(pending.pop(0))
                        tick_finishes()
            while pending:
                drain(pending.pop(0))
            tick_finishes(force=True)

    nc.compile()
    return nc


_NC_CACHE = {}


def kernel(**inputs):
    global LAST_EXEC_NS
    _install_ntff_shim()
    from concourse.bass_utils import run_bass_kernel_spmd

    x = np.asarray(inputs["x"], dtype=np.float32)
    lengths = np.asarray(inputs["lengths"]).astype(np.int64)
    theta_w = np.asarray(inputs["theta_w"], np.float32)
    theta_b = np.asarray(inputs["theta_b"], np.float32)
    phi_w = np.asarray(inputs["phi_w"], np.float32)
    g_w = np.asarray(inputs["g_w"], np.float32)
    g_b = np.asarray(inputs["g_b"], np.float32)
    W_w = np.asarray(inputs["W_w"], np.float32)
    W_b = np.asarray(inputs["W_b"], np.float32)

    bf16 = ml_dtypes.bfloat16
    tw_np = np.ascontiguousarray(theta_w.reshape(2, P, CI)).astype(bf16)
    pw_np = np.ascontiguousarray(phi_w.reshape(2, P, CI)).astype(bf16)
    gw_np = np.ascontiguousarray(g_w.reshape(2, P, CI)).astype(bf16)
    ww_np = np.ascontiguousarray(W_w).astype(bf16)
    tb_np = np.ascontiguousarray(theta_b.reshape(P, 1)).astype(np.float32)
    resid_base = (W_b + g_b @ W_w)[None, :].astype(np.float32)

    lens = [max(0, min(N, int(lengths[b]))) for b in range(B)]
    nkb = max(1, max(-(-L // P) for L in lens))
    keys_processed = nkb * P
    in_maps = []
    for b in range(B):
        L = lens[b]
        rowmask = (np.arange(N) < L).astype(np.float32)
        xz = x[b] * rowmask[:, None]
        xt_np = np.ascontiguousarray(xz.T).reshape(2, P, N).astype(bf16)
        xr_np = np.ascontiguousarray((x[b] + resid_base) * rowmask[:, None]).astype(
            np.float32
        )
        ninv_val = -(keys_processed - L) + (1.0 if L == 0 else 0.0)
        qm2_np = np.ascontiguousarray(rowmask.reshape(NKB, P).T)
        in_maps.append(
            {
                "xt": xt_np,
                "xr": xr_np,
                "tw": tw_np,
                "pw": pw_np,
                "gw": gw_np,
                "ww": ww_np,
                "tb": tb_np,
                "qm2": qm2_np,
                "ninv": np.full((P, 1), ninv_val, np.float32),
            }
        )

    if nkb not in _NC_CACHE:
        _NC_CACHE[nkb] = build(nkb)
    nc = _NC_CACHE[nkb]

    res = run_bass_kernel_spmd(nc, in_maps, list(range(B)))
    LAST_EXEC_NS = res.exec_time_ns
    out = np.stack([np.asarray(res.results[i]["out"]) for i in range(B)]).astype(
        np.float32
    )
    return out


if __name__ == "__main__":
    rng = np.random.default_rng(0)
    demo = {
        "x": rng.standard_normal((B, N, C), dtype=np.float32),
        "lengths": rng.integers(N // 2, N + 1, size=(B,)).astype(np.int32),
        "g_w": (rng.standard_normal((C, CI)) * 0.02).astype(np.float32),
        "g_b": np.zeros(CI, np.float32),
        "theta_w": (rng.standard_normal((C, CI)) * 0.02).astype(np.float32),
        "theta_b": np.zeros(CI, np.float32),
        "phi_w": (rng.standard_normal((C, CI)) * 0.02).astype(np.float32),
        "phi_b": np.zeros(CI, np.float32),
        "W_w": (rng.standard_normal((CI, C)) * 0.02).astype(np.float32),
        "W_b": np.zeros(C, np.float32),
    }
    o = kernel(**demo)
    print("out", o.shape, o.dtype, float(np.abs(o).mean()))


# revision 10
# speedup vs baseline: 1.1947x; 1.1579x over previous
"""Trainium2 Bass kernel for the sparse (ragged) non-local attention block.

Math (per batch b, L = lengths[b], with q/k < N=4096, c < C=256, i < CI=128):
    theta = x @ theta_w + theta_b ; phi = x @ phi_w + phi_b ; g = x @ g_w + g_b
    s[q,k] = theta[q]·phi[k]   (k >= L masked to -inf)
    attn = softmax_k(s) ; y = attn @ g ; z = (y @ W_w + W_b + x) * (q < L)

Sharding: pure data parallel — batch b on core b (8 batches, 8 cores), no
collectives. One static SPMD graph; all raggedness is folded into per-core
host-prepared inputs:
  - xt: x[b]^T in bf16 with columns k >= L zeroed.  Then phi/g columns for
    invalid keys are exactly 0 (phi_b is skipped on-chip: adding phi_b shifts
    every valid key's score by a per-query constant, which softmax cancels).
    Invalid keys thus score s=0, p=exp(0)=1, and contribute p*g=0 to y.
  - ninv = -(N-L): corrects the denominator for those exp(0)=1 terms.
  - xr = (x[b] + W_b + g_b @ W_w) * rowmask: residual with the g_b/W_b biases
    folded in exactly (sum_k attn = 1), zeroed for invalid query rows.
  - qm2: per-(row-block) validity mask, folded into the reciprocal so invalid
    rows emit 0.

Only the first nkb = max_b ceil(L_b/128) query blocks are processed (rows
q >= nkb*128 are masked to zero on every core anyway); the tail output rows
are zero-filled by DMA from a memset tile.

On-chip per core (all matmuls bf16, f32 PSUM accumulation):
  thetaT/phiT [ci,n] projections (theta_b added per-partition), g [k,ci]
  natural-layout projection; then per query chunk (up to 512 wide): for each
  128-key block s^T[k,q] = phiT_kb^T @ thetaT (PE) -> p = exp(s) (ScalarE,
  bf16) -> yT[ci,q] += g_kb^T @ p (PE, PSUM accum) and psb += p (DVE, bf16);
  denom = ones^T @ psb (PE, f32 accum) + ninv; r = qm/denom
  (partition-parallel after a 1->128 spread matmul); per 128-query block
  w = yT_qb^T @ W_w (PE), out = w * r + xr (one fused DVE op) -> DMA out.
"""

import sys

if "/opt/trn_rl_repo" not in sys.path:
    sys.path.insert(0, "/opt/trn_rl_repo")

import contextlib
import ctypes
import types

import ml_dtypes
import numpy as np

import concourse.bass as bass
import concourse.mybir as mybir
import concourse.tile as tile
from concourse import bacc
from concourse.bass import ts

B, N, C, CI = 8, 4096, 256, 128
P = 128
NKB = N // P  # 32 key blocks
QC = 512  # query chunk
GSZ = 3  # key blocks per exp group (3 PSUM banks wide)
NSL = 1  # bf16 p_sum slices

dt = mybir.dt
AF = mybir.ActivationFunctionType
OP = mybir.AluOpType

LAST_EXEC_NS = None


def _install_ntff_shim():
    """Register the axon NTFF profile hook (missing antenv.axon_hooks in this
    image) so run_bass_kernel_spmd(trace=True) can report HW exec time."""
    if "antenv.axon_hooks" in sys.modules:
        return
    try:
        import antenv

        mod = types.ModuleType("antenv.axon_hooks")
        _state = {"hook": None}
        mod.set_axon_ntff_profile_hook = lambda h: _state.__setitem__("hook", h)
        mod.get_axon_ntff_profile_hook = lambda: _state["hook"]
        sys.modules["antenv.axon_hooks"] = mod
        antenv.axon_hooks = mod

        lib = ctypes.CDLL("/opt/axon/libaxon_pjrt.so")
        if not hasattr(lib, "axon_start_nrt_profile"):
            return
        lib.axon_start_nrt_profile.argtypes = [
            ctypes.POINTER(ctypes.c_int64),
            ctypes.c_size_t,
        ]
        lib.axon_start_nrt_profile.restype = ctypes.c_int64
        lib.axon_stop_nrt_profile.argtypes = [ctypes.c_char_p]
        lib.axon_stop_nrt_profile.restype = ctypes.c_int64

        @contextlib.contextmanager
        def _hook(output_dir, device_ids):
            import jax

            jax.devices()
            if device_ids:
                ids = (ctypes.c_int64 * len(device_ids))(*device_ids)
                rc = lib.axon_start_nrt_profile(ids, len(device_ids))
            else:
                rc = lib.axon_start_nrt_profile(None, 0)
            if rc != 0:
                raise RuntimeError(f"axon_start_nrt_profile rc={rc}")
            try:
                yield
            finally:
                n = lib.axon_stop_nrt_profile(str(output_dir).encode())
                if n < 0:
                    raise RuntimeError(f"axon_stop_nrt_profile rc={n}")

        mod.set_axon_ntff_profile_hook(_hook)
    except Exception:
        pass


def _enable_ldw_opt():
    """Flip walrus --enable-ldw-opt to true (overlaps LDWEIGHTS with matmul
    streaming via the background weight buffer)."""
    from concourse import bass_utils as bu

    if getattr(bu, "_ldw_patched", False):
        return
    orig = bu.run_command

    def patched(cmd, *a, **kw):
        if isinstance(cmd, list):
            cmd = [
                "--enable-ldw-opt=true" if c == "--enable-ldw-opt=false" else c
                for c in cmd
            ]
        return orig(cmd, *a, **kw)

    bu.run_command = patched
    bu._ldw_patched = True


def build(nkb):
    # nkb = number of 128-wide blocks actually processed on both the key and
    # the query axis (= max over cores of ceil(L/128)); key blocks beyond it
    # are fully masked on every core (host denominator correction counts only
    # processed keys) and query rows beyond it are zero on every core.
    groups = []
    _kb = 0
    while _kb < nkb:
        g = min(GSZ, nkb - _kb)
        groups.append((_kb, g))
        _kb += g
    # query chunks: (start_block, n_blocks)
    chunks = []
    _qb = 0
    while _qb < nkb:
        nb = min(QC // P, nkb - _qb)
        chunks.append((_qb, nb))
        _qb += nb
    nslices = min(GSZ, nkb)  # psb partial-sum slices (one per group lane)
    nc = bacc.Bacc("TRN2", target_bir_lowering=False, debug=False, num_devices=B)

    xt = nc.declare_dram_parameter("xt", [2, P, N], dt.bfloat16, False)
    xr = nc.declare_dram_parameter("xr", [N, C], dt.float32, False)
    tw = nc.declare_dram_parameter("tw", [2, P, CI], dt.bfloat16, False)
    pw = nc.declare_dram_parameter("pw", [2, P, CI], dt.bfloat16, False)
    gw = nc.declare_dram_parameter("gw", [2, P, CI], dt.bfloat16, False)
    ww = nc.declare_dram_parameter("ww", [CI, C], dt.bfloat16, False)
    tb = nc.declare_dram_parameter("tb", [P, 1], dt.float32, False)
    qm2 = nc.declare_dram_parameter("qm2", [P, NKB], dt.float32, False)
    ninv = nc.declare_dram_parameter("ninv", [P, 1], dt.float32, False)
    out = nc.declare_dram_parameter("out", [N, C], dt.float32, True)

    nq = nkb * P  # processed queries / keys

    with tile.TileContext(nc) as tc:
        with (
            tc.tile_pool(name="wpool", bufs=1) as wpool,
            tc.tile_pool(name="xtp", bufs=1) as xtp,
            tc.tile_pool(name="feat", bufs=1) as feat,
            tc.tile_pool(name="ppool", bufs=3) as ppool,
            tc.tile_pool(name="psbp", bufs=2) as psbp,
            tc.tile_pool(name="ysbp", bufs=2) as ysbp,
            tc.tile_pool(name="smallp", bufs=2) as smallp,
            tc.tile_pool(name="xrp", bufs=8) as xrp,
            tc.tile_pool(name="outp", bufs=4) as outp,
            tc.tile_pool(name="sc_ps", bufs=2, space="PSUM") as sc_ps,
            tc.tile_pool(name="y_ps", bufs=2, space="PSUM") as y_ps,
        ):
            # ---- constants / weights to SBUF ----
            tw_s = wpool.tile([P, 2 * CI], dt.bfloat16, tag="tw")
            pw_s = wpool.tile([P, 2 * CI], dt.bfloat16, tag="pw")
            gw_s = wpool.tile([P, 2 * CI], dt.bfloat16, tag="gw")
            for i in range(2):
                nc.sync.dma_start(tw_s[:, ts(i, CI)], tw.ap()[i])
                nc.sync.dma_start(pw_s[:, ts(i, CI)], pw.ap()[i])
                nc.sync.dma_start(gw_s[:, ts(i, CI)], gw.ap()[i])
            ww_s = wpool.tile([CI, C], dt.bfloat16, tag="ww")
            nc.sync.dma_start(ww_s[:], ww.ap()[:])
            tb_s = wpool.tile([P, 1], dt.float32, tag="tb")
            nc.sync.dma_start(tb_s[:], tb.ap()[:])
            qm_s = wpool.tile([P, NKB], dt.float32, tag="qm")
            nc.sync.dma_start(qm_s[:], qm2.ap()[:])
            ninv_s = wpool.tile([P, 1], dt.float32, tag="ninv")
            nc.sync.dma_start(ninv_s[:], ninv.ap()[:])
            ones_s = wpool.tile([P, 1], dt.bfloat16, tag="ones")
            nc.vector.memset(ones_s[:], 1.0)

            # zero-fill tile for unprocessed query rows
            if nkb < NKB:
                z_s = wpool.tile([P, C], dt.float32, tag="z")
                nc.vector.memset(z_s[:], 0.0)
                for qb in range(nkb, NKB):
                    nc.sync.dma_start(out.ap()[qb * P : (qb + 1) * P, :], z_s[:])

            # xt is loaded in per-chunk pieces so the first projection matmuls
            # can start as soon as their slice lands (kills ~20us of startup
            # serialization on the full 2MB transfer).
            xt_s = xtp.tile([P, 2 * N], dt.bfloat16, tag="xt")
            for qb0, nb in chunks:
                w = nb * P
                c0 = qb0 * P
                for i in range(2):
                    nc.sync.dma_start(
                        xt_s[:, i * N + c0 : i * N + c0 + w],
                        xt.ap()[i, :, c0 : c0 + w],
                    )
            if nkb < NKB:
                # remaining columns are never read; no need to load them
                pass

            # ---- projections (only the first nq columns are needed) ----
            thetaT = feat.tile([P, N], dt.bfloat16, tag="thetaT")
            phiT = feat.tile([P, N], dt.bfloat16, tag="phiT")
            g_s = feat.tile([P, N], dt.bfloat16, tag="g")

            for ch, (qb0, nb) in enumerate(chunks):
                w = nb * P
                c0 = qb0 * P
                pth = sc_ps.tile([P, GSZ, QC], dt.float32, tag="sc", name="pth")
                nc.tensor.matmul(
                    pth[:, 0, :w],
                    lhsT=tw_s[:, 0:CI],
                    rhs=xt_s[:, c0 : c0 + w],
                    start=True,
                    stop=False,
                )
                nc.tensor.matmul(
                    pth[:, 0, :w],
                    lhsT=tw_s[:, CI : 2 * CI],
                    rhs=xt_s[:, N + c0 : N + c0 + w],
                    start=False,
                    stop=True,
                )
                nc.vector.tensor_scalar_add(
                    thetaT[:, c0 : c0 + w], pth[:, 0, :w], tb_s[:, 0:1]
                )
                pph = sc_ps.tile([P, GSZ, QC], dt.float32, tag="sc", name="pph")
                nc.tensor.matmul(
                    pph[:, 0, :w],
                    lhsT=pw_s[:, 0:CI],
                    rhs=xt_s[:, c0 : c0 + w],
                    start=True,
                    stop=False,
                )
                nc.tensor.matmul(
                    pph[:, 0, :w],
                    lhsT=pw_s[:, CI : 2 * CI],
                    rhs=xt_s[:, N + c0 : N + c0 + w],
                    start=False,
                    stop=True,
                )
                nc.vector.tensor_copy(phiT[:, c0 : c0 + w], pph[:, 0, :w])

            for kbp in range(-(-nkb // 2)):
                pg = sc_ps.tile([P, GSZ, QC], dt.float32, tag="sc", name="pg")
                nh = min(2, nkb - kbp * 2)
                for h in range(nh):
                    kb = kbp * 2 + h
                    nc.tensor.matmul(
                        pg[:, h, 0:CI],
                        lhsT=xt_s[:, kb * P : (kb + 1) * P],
                        rhs=gw_s[:, 0:CI],
                        start=True,
                        stop=False,
                    )
                    nc.tensor.matmul(
                        pg[:, h, 0:CI],
                        lhsT=xt_s[:, N + kb * P : N + (kb + 1) * P],
                        rhs=gw_s[:, CI : 2 * CI],
                        start=False,
                        stop=True,
                    )
                nc.vector.tensor_copy(
                    g_s[:, kbp * 2 * P : kbp * 2 * P + nh * P].rearrange(
                        "p (h c) -> p h c", h=nh
                    ),
                    pg[:, 0:nh, 0:CI],
                )

            # ---- attention, software-pipelined one exp-group deep ----
            qstate = {}

            def start_qc(qc):
                qb0, nb = chunks[qc]
                ysum = y_ps.tile([P, QC], dt.float32, tag="ysum", name="ysum")
                psb = psbp.tile([P, GSZ, QC], dt.bfloat16, tag="psb", name="psb")
                xr_ts = []
                for j in range(nb):
                    qb = qb0 + j
                    xr_t = xrp.tile([P, C], dt.float32, tag="xr", name="xr_t")
                    nc.sync.dma_start(xr_t[:], xr.ap()[qb * P : (qb + 1) * P, :])
                    xr_ts.append(xr_t)
                qstate[qc] = (ysum, psb, xr_ts)

            def drain(item):
                qc, kb0, gsz, p = item
                qb0, nb = chunks[qc]
                w = nb * P
                if kb0 == 0:
                    start_qc(qc)
                ysum, psb, xr_ts = qstate[qc]
                for j in range(gsz):
                    kbj = kb0 + j
                    nc.tensor.matmul(
                        ysum[:, :w],
                        lhsT=g_s[:, ts(kbj, P)],
                        rhs=p[:, j, :w],
                        start=(kbj == 0),
                        stop=(kbj == nkb - 1),
                        skip_group_check=True,
                    )
                # one wide DVE op accumulates the whole exp-group into the
                # per-lane partial sums (slice s gets key blocks kb0+s)
                if kb0 == 0:
                    nc.vector.tensor_copy(psb[:, :gsz, :w], p[:, :gsz, :w])
                else:
                    nc.vector.tensor_add(
                        psb[:, :gsz, :w], psb[:, :gsz, :w], p[:, :gsz, :w]
                    )
                if kb0 + gsz == nkb:
                    finish_queue.append([qc, 0])

            def finish_qc(qc):
                qb0, nb = chunks[qc]
                w = nb * P
                ysum, psb, xr_ts = qstate.pop(qc)
                # fold the partial-sum slices into slice 0, then reduce over
                # keys per 128-query block with psb as the matmul stationary:
                # out = psb_block.T @ ones lands as [q-partition, 1] directly
                # (no 1->128 spread needed).
                for s in range(1, nslices):
                    nc.vector.tensor_add(
                        psb[:, 0, :w], psb[:, 0, :w], psb[:, s, :w]
                    )
                ds = sc_ps.tile([P, GSZ, QC], dt.float32, tag="sc", name="ds")
                for j in range(nb):
                    nc.tensor.matmul(
                        ds[:, 0, j : j + 1],
                        lhsT=psb[:, 0, ts(j, P)],
                        rhs=ones_s[:, 0:1],
                        start=True,
                        stop=True,
                        skip_group_check=True,
                    )
                dn = smallp.tile([P, 4], dt.float32, tag="dn", name="dn")
                nc.vector.tensor_scalar_add(
                    dn[:, :nb], ds[:, 0, 0:nb], ninv_s[:, 0:1]
                )
                rc = smallp.tile([P, 4], dt.float32, tag="rc", name="rc")
                nc.vector.reciprocal(rc[:, :nb], dn[:, :nb])
                r_t = smallp.tile([P, 4], dt.float32, tag="rt", name="r_t")
                nc.vector.tensor_mul(
                    r_t[:, :nb], rc[:, :nb], qm_s[:, qb0 : qb0 + nb]
                )
                y_sb = ysbp.tile([P, QC], dt.bfloat16, tag="ysb", name="y_sb")
                nc.vector.tensor_copy(y_sb[:, :w], ysum[:, :w])
                for j in range(nb):
                    qb = qb0 + j
                    wy = sc_ps.tile([P, GSZ, QC], dt.float32, tag="sc", name="wy")
                    nc.tensor.matmul(
                        wy[:, 0, 0:C],
                        lhsT=y_sb[:, ts(j, P)],
                        rhs=ww_s[:],
                        start=True,
                        stop=True,
                    )
                    ot = outp.tile([P, C], dt.float32, tag="ot", name="ot")
                    nc.vector.scalar_tensor_tensor(
                        ot[:],
                        wy[:, 0, 0:C],
                        r_t[:, j : j + 1],
                        xr_ts[j][:],
                        OP.mult,
                        OP.add,
                    )
                    nc.sync.dma_start(out.ap()[qb * P : (qb + 1) * P, :], ot[:])

            pending = []
            finish_queue = []

            def tick_finishes(force=False):
                for ent in list(finish_queue):
                    ent[1] += 1
                    if force or ent[1] > 2:
                        finish_qc(ent[0])
                        finish_queue.remove(ent)

            for qc, (qb0, nb) in enumerate(chunks):
                w = nb * P
                c0 = qb0 * P
                for kb0, gsz in groups:
                    sc = sc_ps.tile([P, GSZ, QC], dt.float32, tag="sc", name="sc")
                    for j in range(gsz):
                        nc.tensor.matmul(
                            sc[:, j, :w],
                            lhsT=phiT[:, ts(kb0 + j, P)],
                            rhs=thetaT[:, c0 : c0 + w],
                            start=True,
                            stop=True,
                        )
                    p = ppool.tile([P, GSZ, QC], dt.bfloat16, tag="p", name="p")
                    nc.scalar.activation(p[:, :gsz, :w], sc[:, :gsz, :w], AF.Exp)
                    pending.append((qc, kb0, gsz, p))
                    if len(pending) > 1:
                        drain(pending.pop(0))
                        tick_finishes()
            while pending:
                drain(pending.pop(0))
            tick_finishes(force=True)

    nc.compile()
    return nc


_NC_CACHE = {}


def kernel(**inputs):
    global LAST_EXEC_NS
    _install_ntff_shim()
    from concourse.bass_utils import run_bass_kernel_spmd

    x = np.asarray(inputs["x"], dtype=np.float32)
    lengths = np.asarray(inputs["lengths"]).astype(np.int64)
    theta_w = np.asarray(inputs["theta_w"], np.float32)
    theta_b = np.asarray(inputs["theta_b"], np.float32)
    phi_w = np.asarray(inputs["phi_w"], np.float32)
    g_w = np.asarray(inputs["g_w"], np.float32)
    g_b = np.asarray(inputs["g_b"], np.float32)
    W_w = np.asarray(inputs["W_w"], np.float32)
    W_b = np.asarray(inputs["W_b"], np.float32)

    bf16 = ml_dtypes.bfloat16
    tw_np = np.ascontiguousarray(theta_w.reshape(2, P, CI)).astype(bf16)
    pw_np = np.ascontiguousarray(phi_w.reshape(2, P, CI)).astype(bf16)
    gw_np = np.ascontiguousarray(g_w.reshape(2, P, CI)).astype(bf16)
    ww_np = np.ascontiguousarray(W_w).astype(bf16)
    tb_np = np.ascontiguousarray(theta_b.reshape(P, 1)).astype(np.float32)
    resid_base = (W_b + g_b @ W_w)[None, :].astype(np.float32)

    lens = [max(0, min(N, int(lengths[b]))) for b in range(B)]
    nkb = max(1, max(-(-L // P) for L in lens))
    keys_processed = nkb * P
    in_maps = []
    for b in range(B):
        L = lens[b]
        rowmask = (np.arange(N) < L).astype(np.float32)
        xz = x[b] * rowmask[:, None]
        xt_np = np.ascontiguousarray(xz.T).reshape(2, P, N).astype(bf16)
        xr_np = np.ascontiguousarray((x[b] + resid_base) * rowmask[:, None]).astype(
            np.float32
        )
        ninv_val = -(keys_processed - L) + (1.0 if L == 0 else 0.0)
        qm2_np = np.ascontiguousarray(rowmask.reshape(NKB, P).T)
        in_maps.append(
            {
                "xt": xt_np,
                "xr": xr_np,
                "tw": tw_np,
                "pw": pw_np,
                "gw": gw_np,
                "ww": ww_np,
                "tb": tb_np,
                "qm2": qm2_np,
                "ninv": np.full((P, 1), ninv_val, np.float32),
            }
        )

    if nkb not in _NC_CACHE:
        _NC_CACHE[nkb] = build(nkb)
    nc = _NC_CACHE[nkb]

    res = run_bass_kernel_spmd(nc, in_maps, list(range(B)))
    LAST_EXEC_NS = res.exec_time_ns
    out = np.stack([np.asarray(res.results[i]["out"]) for i in range(B)]).astype(
        np.float32
    )
    return out


if __name__ == "__main__":
    rng = np.random.default_rng(0)
    demo = {
        "x": rng.standard_normal((B, N, C), dtype=np.float32),
        "lengths": rng.integers(N // 2, N + 1, size=(B,)).astype(np.int32),
        "g_w": (rng.standard_normal((C, CI)) * 0.02).astype(np.float32),
        "g_b": np.zeros(CI, np.float32),
        "theta_w": (rng.standard_normal((C, CI)) * 0.02).astype(np.float32),
        "theta_b": np.zeros(CI, np.float32),
        "phi_w": (rng.standard_normal((C, CI)) * 0.02).astype(np.float32),
        "phi_b": np.zeros(CI, np.float32),
        "W_w": (rng.standard_normal((CI, C)) * 0.02).astype(np.float32),
        "W_b": np.zeros(C, np.float32),
    }
    o = kernel(**demo)
    print("out", o.shape, o.dtype, float(np.abs(o).mean()))
